# revision 1
# baseline (speedup 1.0000x reference)
"""DiT block kernel for 8x Trainium2 NeuronCores (Bass/Tile).

Sharding: row-parallel over the flattened (B,T)=4096 rows; 512 rows/core.
Cores 0-3 handle batch 0, cores 4-7 batch 1. MQA K/V is computed per-shard
and AllGather'd within each 4-core batch group. Weights are replicated and
cast to bf16 (PE runs bf16 at 1 cycle/row); LN/residual math stays fp32.

Device-side layout notes (per core, R=512 rows):
  - rows-on-partitions for LN/residual tensors (bn_stats reduces over free)
  - hn/h2 are PE-transposed to hT [F-tile, rows] to serve as matmul rhs
  - attention scores are computed transposed ([keys, rows]) so the exp'd
    probs tiles can be used directly as lhsT of the PV matmul; a ones
    column appended to V yields softmax denominators for free; the 1/sum
    is a per-partition scalar on the untransposed PV output.
  - SBUF is tight: phase-0 temporaries and attention-era tiles live in
    pools that are closed as soon as their phase ends.
"""

import os
import sys

sys.path.insert(0, "/opt/trn_rl_repo")

import numpy as np
import ml_dtypes

BF16 = ml_dtypes.bfloat16

B, T, F, H, D, M, C = 2, 2048, 1024, 16, 64, 4, 512
NCORES = 8
R = (B * T) // NCORES  # 512 rows per core
RB = R // 128  # 4 row blocks
FT = F // 128  # 8 feature tiles
MT = (H * D) // 128  # 8 head-pair tiles
MFT = (M * F) // 128  # 32 mlp hidden tiles
KT = T // 128  # 16 key tiles
EPS = 1e-5

_CACHE = {}


def _build_nc():
    import concourse.bass as bass
    import concourse.tile as tile
    from concourse import bacc, mybir
    from concourse.masks import make_identity
    from contextlib import ExitStack

    f32 = mybir.dt.float32
    f16 = mybir.dt.float16
    bf16 = mybir.dt.bfloat16
    AF = mybir.ActivationFunctionType
    OP = mybir.AluOpType

    nc = bacc.Bacc(
        "TRN2",
        target_bir_lowering=False,
        debug=False,
        enable_asserts=False,
        num_devices=NCORES,
    )

    def dram(name, shape, dt, **kw):
        return nc.dram_tensor(name, shape, dt, **kw).ap()

    x_d = dram("x", [R, F], f32, kind="ExternalInput")
    cond_d = dram("cond", [C], bf16, kind="ExternalInput")
    wmod_d = dram("wmod", [C, 4 * F], bf16, kind="ExternalInput")
    modb_d = dram("modb", [4 * F], f32, kind="ExternalInput")
    lnv_d = dram("lnvec", [6, F], f32, kind="ExternalInput")
    wq_d = dram("wq", [MT, 128, FT * 128], bf16, kind="ExternalInput")
    wkv_d = dram("wkv", [F, 2 * D], bf16, kind="ExternalInput")
    wo_d = dram("wo", [H * D, F], bf16, kind="ExternalInput")
    wob_d = dram("wo_bias", [1, F], f32, kind="ExternalInput")
    w1_d = dram("w1", [MFT, 128, FT * 128], bf16, kind="ExternalInput")
    b1_d = dram("b1", [M * F], f32, kind="ExternalInput")
    w2_d = dram("w2", [M * F, F], bf16, kind="ExternalInput")
    b2_d = dram("b2", [1, F], f32, kind="ExternalInput")
    y_d = dram("y", [R, F], f32, kind="ExternalOutput")

    groups = [[0, 1, 2, 3], [4, 5, 6, 7]]

    def bcast_row(ap_row):
        # [1, n] DRAM AP -> partition-broadcast [128, n]
        return bass.AP(
            tensor=ap_row.tensor,
            offset=ap_row.offset,
            ap=[[0, 128]] + list(ap_row.ap[-1:]),
        )

    with tile.TileContext(nc) as tc, ExitStack() as ctx:
        consts = ctx.enter_context(tc.tile_pool(name="consts", bufs=1))
        work = ctx.enter_context(tc.tile_pool(name="work", bufs=2))
        persist = ctx.enter_context(tc.tile_pool(name="persist", bufs=1))
        wstr = ctx.enter_context(tc.tile_pool(name="wstr", bufs=3))
        dramp = ctx.enter_context(tc.tile_pool(name="dramp", bufs=1, space="DRAM"))
        psA = ctx.enter_context(tc.tile_pool(name="psA", bufs=4, space="PSUM"))
        psB = ctx.enter_context(tc.tile_pool(name="psB", bufs=2, space="PSUM"))

        _dmaq_state = [0]

        def dmaq(out, in_):
            # alternate big transfers across the two HWDGE queues
            eng = nc.sync if _dmaq_state[0] % 2 == 0 else nc.scalar
            _dmaq_state[0] += 1
            eng.dma_start(out=out, in_=in_)

        # ---------------- constants ----------------
        ident = consts.tile([128, 128], bf16, name="ident")
        make_identity(nc, ident)
        ones16 = consts.tile([1, 128], f16, name="ones16")
        nc.vector.memset(ones16, 1.0)
        epst = consts.tile([128, 1], f32, name="epst")
        nc.vector.memset(epst, EPS)

        cond_sb = consts.tile([128, 4], bf16, name="cond_sb")
        nc.sync.dma_start(out=cond_sb, in_=cond_d.rearrange("(a p) -> p a", p=128))
        b1_sb = consts.tile([128, MFT], f32, name="b1_sb")
        nc.sync.dma_start(out=b1_sb, in_=b1_d.rearrange("(mt p) -> p mt", p=128))
        wkv_sb = consts.tile([128, FT, 2 * D], bf16, name="wkv_sb")
        nc.sync.dma_start(
            out=wkv_sb, in_=wkv_d.rearrange("(kt p) n -> p kt n", p=128)
        )

        anw_bc = consts.tile([128, F], f32, name="anw_bc")
        nc.sync.dma_start(out=anw_bc, in_=bcast_row(lnv_d[2:3, :]))
        anb_bc = consts.tile([128, F], f32, name="anb_bc")
        nc.sync.dma_start(out=anb_bc, in_=bcast_row(lnv_d[3:4, :]))
        wob_bc = consts.tile([128, F], f32, name="wob_bc")
        nc.sync.dma_start(out=wob_bc, in_=bcast_row(wob_d[0:1, :]))
        b2_bc = consts.tile([128, F], f32, name="b2_bc")
        nc.sync.dma_start(out=b2_bc, in_=bcast_row(b2_d[0:1, :]))

        # ---------------- phase 0: modulation vectors ----------------
        # modv = cond @ [gA | bA | gF | bF] + modb  -> [1, 4F] fp32, then
        # Wa = amod_nw*(1+gA), Ba = amod_nb*(1+gA)+bA (same for fmod),
        # PE-broadcast to [128, F] fp32 tiles.
        cm_modtmp = tc.tile_pool(name="modtmp", bufs=1)
        modtmp = cm_modtmp.__enter__()

        lnr = {}
        for r in (0, 1, 4, 5):  # amod_nw/nb, fmod_nw/nb rows at partition 0
            lnr[r] = modtmp.tile([1, F], f32, name=f"lnr{r}")
            nc.sync.dma_start(out=lnr[r], in_=lnv_d[r : r + 1, :])
        modb_sb = modtmp.tile([1, 4 * F], f32, name="modb_sb")
        nc.sync.dma_start(out=modb_sb, in_=modb_d.rearrange("(a f) -> a f", a=1))
        modv = modtmp.tile([1, 4 * F], f32, name="modv")
        for grp in range(2):  # nb groups of 4 -> 4 concurrent psum accumulators
            wm_tiles = []
            for ch in range(4):
                wm = modtmp.tile(
                    [128, 2048], bf16, tag="wm", bufs=2, name=f"wm{grp}_{ch}"
                )
                dmaq(
                    wm,
                    wmod_d[ch * 128 : (ch + 1) * 128, grp * 2048 : (grp + 1) * 2048],
                )
                wm_tiles.append(wm)
            pms = [
                psA.tile([128, 512], f32, tag="ps", name=f"pm{j}") for j in range(4)
            ]
            for ch in range(4):
                for j in range(4):
                    nc.tensor.matmul(
                        pms[j][0:1, :],
                        cond_sb[:, ch : ch + 1],
                        wm_tiles[ch][:, j * 512 : (j + 1) * 512],
                        start=(ch == 0),
                        stop=(ch == 3),
                    )
            for j in range(4):
                nb = grp * 4 + j
                nc.vector.tensor_add(
                    out=modv[:, nb * 512 : (nb + 1) * 512],
                    in0=pms[j][0:1, :],
                    in1=modb_sb[:, nb * 512 : (nb + 1) * 512],
                )

        # in-place per mod: g-slot := nw*(1+g), b-slot := nb*(1+g) + b, then
        # PE-broadcast. amod (grp0 columns) is finalized first so adaLN-1
        # can start before the fmod half of wmod has even arrived.
        tmpv = modtmp.tile([1, F], f32, name="tmpv")
        bc = {}

        modv16 = modtmp.tile([1, 4 * F], f16, name="modv16")

        def finalize_mod(g_off, b_off, nw_row, nb_row, w_name, b_name):
            g_sl = modv[:, g_off : g_off + F]
            b_sl = modv[:, b_off : b_off + F]
            nc.scalar.add(out=g_sl, in_=g_sl, add=1.0)
            nc.vector.tensor_mul(out=tmpv, in0=g_sl, in1=lnr[nb_row])
            with nc.allow_low_precision(reason="f16 staging for PE broadcast"):
                nc.vector.tensor_add(
                    out=modv16[:, b_off : b_off + F], in0=tmpv, in1=b_sl
                )
                nc.vector.tensor_mul(
                    out=modv16[:, g_off : g_off + F], in0=g_sl, in1=lnr[nw_row]
                )
            for off, nm in ((g_off, w_name), (b_off, b_name)):
                bt = consts.tile([128, F], f32, name=nm)
                for hf in range(2):
                    pb = psA.tile([128, 512], f32, tag="ps", name="pbc")
                    nc.tensor.matmul(
                        pb,
                        ones16,
                        modv16[:, off + hf * 512 : off + (hf + 1) * 512],
                        start=True,
                        stop=True,
                    )
                    nc.scalar.activation(
                        bt[:, hf * 512 : (hf + 1) * 512], pb, AF.Copy
                    )
                bc[nm] = bt

        finalize_mod(0, F, 0, 1, "Wa_bc", "Ba_bc")
        finalize_mod(2 * F, 3 * F, 4, 5, "Wf_bc", "Bf_bc")

        cm_modtmp.__exit__(None, None, None)

        # ---------------- helpers ----------------
        def layer_norm(src, w_bc, b_bc, out_tile):
            """out = LN(src) * w_bc + b_bc ; src [128,F] f32."""
            stats = work.tile([128, 2, 6], f32, tag="stats", name="stats")
            for sg in range(2):
                nc.vector.bn_stats(
                    out=stats[:, sg, :], in_=src[:, sg * 512 : (sg + 1) * 512]
                )
            mv = work.tile([128, 2], f32, tag="mv", name="mv")
            nc.vector.bn_aggr(out=mv, in_=stats)
            rstd = work.tile([128, 1], f32, tag="rstd", name="rstd")
            nc.scalar.activation(
                out=rstd, in_=mv[:, 1:2], func=AF.Sqrt, bias=epst, scale=1.0
            )
            nc.vector.reciprocal(out=rstd, in_=rstd)
            xn = work.tile([128, F], f32, tag="xn", name="xn")
            nc.vector.tensor_scalar(
                out=xn,
                in0=src,
                scalar1=mv[:, 0:1],
                scalar2=rstd,
                op0=OP.subtract,
                op1=OP.mult,
            )
            nc.vector.tensor_mul(out=xn, in0=xn, in1=w_bc)
            # final add on GpSimd frees DVE for the next row-block's stats
            nc.gpsimd.tensor_add(out=out_tile, in0=xn, in1=b_bc)

        def transpose_to(hsrc_bf, hT_tiles, rb):
            """hsrc_bf [128,F] bf16 -> hT_tiles[ft][:, rb*128:+128]."""
            for ft in range(FT):
                pt = psA.tile([128, 128], bf16, tag="ps", name="ptt")
                nc.tensor.transpose(
                    pt, hsrc_bf[:, ft * 128 : (ft + 1) * 128], ident
                )
                nc.scalar.activation(
                    out=hT_tiles[ft][:, rb * 128 : (rb + 1) * 128],
                    in_=pt,
                    func=AF.Copy,
                )

        # hT tags are reused for h2T in phase 6 (same shape/dtype).
        hT = [
            persist.tile([128, R], bf16, tag=f"hT{ft}", name=f"hT{ft}")
            for ft in range(FT)
        ]

        cm_hera = tc.tile_pool(name="hera", bufs=1)
        hera = cm_hera.__enter__()
        cm_aera = tc.tile_pool(name="aera", bufs=1)
        aera = cm_aera.__enter__()
        cm_attnp = tc.tile_pool(name="attnp", bufs=1)
        attnp = cm_attnp.__enter__()

        # ---------------- phase 1: adaLN-1 + attn-LN + transpose ----------------
        h_res = [hera.tile([128, F], f32, name=f"h{rb}") for rb in range(RB)]
        for rb in range(RB):
            x_rb = work.tile([128, F], f32, tag="x", name="x_rb")
            nc.sync.dma_start(out=x_rb, in_=x_d[rb * 128 : (rb + 1) * 128, :])
            layer_norm(x_rb, bc["Wa_bc"], bc["Ba_bc"], h_res[rb])
            hn_bf = work.tile([128, F], bf16, tag="hnbf", bufs=1, name="hn_bf")
            layer_norm(h_res[rb], anw_bc, anb_bc, hn_bf)
            transpose_to(hn_bf, hT, rb)

        # ---------------- phase 2: kv first (AllGather ASAP), then q ----------------
        pkv = psA.tile([128, 512], f32, tag="ps", name="pkv")
        for kt in range(FT):
            nc.tensor.matmul(
                pkv, wkv_sb[:, kt, :], hT[kt], start=(kt == 0), stop=(kt == FT - 1)
            )
        kvT_sb = work.tile([128, R], bf16, tag="kvT", bufs=1, name="kvT_sb")
        nc.scalar.activation(out=kvT_sb, in_=pkv, func=AF.Copy)

        kvT_bounce = dramp.tile([2 * D, R], bf16, name="kvT_bounce")
        kvT_all = dramp.tile([4 * 2 * D, R], bf16, name="kvT_all")
        nc.sync.dma_start(out=kvT_bounce, in_=kvT_sb)
        nc.gpsimd.collective_compute(
            "AllGather",
            OP.bypass,
            replica_groups=groups,
            ins=[kvT_bounce[:, :]],
            outs=[kvT_all[:, :]],
        )

        # q projection fills the AllGather wait
        qT = [aera.tile([128, R], bf16, name=f"qT{mt}") for mt in range(MT)]
        for mt in range(MT):
            wqblk = wstr.tile([128, FT * 128], bf16, tag="wqb", bufs=2, name="wqblk")
            dmaq(out=wqblk, in_=wq_d[mt])
            pq = psA.tile([128, 512], f32, tag="ps", name="pq")
            for kt in range(FT):
                nc.tensor.matmul(
                    pq,
                    wqblk[:, kt * 128 : (kt + 1) * 128],
                    hT[kt],
                    start=(kt == 0),
                    stop=(kt == FT - 1),
                )
            # fold the attention 1/sqrt(D)=0.125 scale into q
            nc.scalar.activation(out=qT[mt], in_=pq, func=AF.Copy, scale=0.125)

        # ---------------- phase 3: kT / v_ext assembly ----------------
        # k^T duplicated into both partition halves so MM1's lhsT can share
        # the rhs (q head slice) base partition for even and odd heads.
        kT = aera.tile([128, T], bf16, name="kT")
        for hp in (0, 64):
            for r in range(4):
                nc.sync.dma_start(
                    out=kT[hp : hp + 64, r * R : (r + 1) * R],
                    in_=kvT_all[r * 128 : r * 128 + 64, :],
                )
        v_ext = [aera.tile([128, 65], bf16, name=f"vext{kt}") for kt in range(KT)]
        for kt in range(KT):
            nc.vector.memset(v_ext[kt][:, 64:65], 1.0)
        for r in range(4):
            vT_sb = work.tile([64, R], bf16, tag="vTs", bufs=1, name="vT_sb")
            nc.sync.dma_start(
                out=vT_sb, in_=kvT_all[r * 128 + 64 : (r + 1) * 128, :]
            )
            for cc in range(4):
                ptv = psA.tile([128, 128], bf16, tag="ps", name="ptv")
                nc.tensor.matmul(
                    ptv[:, 0:64],
                    vT_sb[:, cc * 128 : (cc + 1) * 128],
                    ident[0:64, 0:64],
                    is_transpose=True,
                )
                nc.scalar.activation(
                    out=v_ext[r * 4 + cc][:, 0:64], in_=ptv[:, 0:64], func=AF.Copy
                )

        # ---------------- phase 4: attention ----------------
        # ones row at partition 64 for the denominator-broadcast matmul
        # (matmul operand base partitions must match; psum sums sit at 64).
        ones64 = consts.tile([128, 64], f16, name="ones64")
        nc.vector.memset(ones64[64:65, :], 1.0)
        outT = [aera.tile([64, R], bf16, name=f"outTh{h}") for h in range(H)]

        def mm1_half(hi, lo, hi_kt, prs):
            """QK^T + exp for key tiles [lo, hi_kt) of head hi."""
            mt, hp = hi // 2, (hi % 2) * 64
            for kt in range(lo, hi_kt):
                ps_s = psA.tile([128, 512], f32, tag="ps", name="ps_s")
                nc.tensor.matmul(
                    ps_s,
                    kT[hp : hp + 64, kt * 128 : (kt + 1) * 128],
                    qT[mt][hp : hp + 64, :],
                    start=True,
                    stop=True,
                )
                pr = attnp.tile(
                    [128, R], bf16, tag=f"pr{kt}", bufs=2 if kt < 8 else 1,
                    name=f"pr{kt}",
                )
                nc.scalar.activation(out=pr, in_=ps_s, func=AF.Exp)
                prs[kt] = pr

        # software pipeline: kt 0..7 of head h+1 (double-buffered probs) are
        # issued ahead; kt 8..15 (single-buffered) after the previous head's
        # PV matmul has consumed them.
        probs_cur: dict = {}
        probs_nxt: dict = {}
        mm1_half(0, 0, 8, probs_cur)
        for hi in range(H):
            probs = probs_cur
            mt, hp = hi // 2, (hi % 2) * 64
            # PV matmul, transposed: out^T[65, rows] accumulated over key
            # tiles; row 64 is the softmax denominator (ones column of v).
            po = psB.tile([128, 512], f32, tag="pb", name="po")
            for kt in range(8):
                nc.tensor.matmul(
                    po[0:65, :],
                    v_ext[kt][:, 0:65],
                    probs[kt],
                    start=(kt == 0),
                    stop=False,
                )
            mm1_half(hi, 8, KT, probs_cur)
            for kt in range(8, KT):
                nc.tensor.matmul(
                    po[0:65, :],
                    v_ext[kt][:, 0:65],
                    probs[kt],
                    start=False,
                    stop=(kt == KT - 1),
                )
            rcp_row = work.tile([128, R], f16, tag="rcp", bufs=1, name="rcp_row")
            with nc.allow_low_precision(reason="f16 softmax denom broadcast"):
                nc.vector.reciprocal(out=rcp_row[64:65, :], in_=po[64:65, :])
            bcr = psB.tile([128, 512], f32, tag="pb", name="bcr")
            nc.tensor.matmul(
                bcr[0:64, :],
                ones64[64:65, :],
                rcp_row[64:65, :],
                start=True,
                stop=True,
            )
            t_sb = work.tile([64, R], bf16, tag="tsb", name="t_sb")
            nc.vector.tensor_copy(out=t_sb, in_=po[0:64, :])
            nc.vector.tensor_mul(out=outT[hi], in0=t_sb, in1=bcr[0:64, :])
            if hi + 1 < H:
                probs_nxt = {}
                mm1_half(hi + 1, 0, 8, probs_nxt)
                probs_cur = probs_nxt

        # ---------------- phase 5: out proj + residual -> x1 ----------------
        x1 = [persist.tile([128, F], f32, name=f"x1_{rt}") for rt in range(RB)]
        for rh in range(2):
            px1 = {}
            for rt in (2 * rh, 2 * rh + 1):
                px1[rt] = psB.tile([128, F], f32, tag="pb", name=f"px1_{rt}")
            for hk in range(H):
                woc = wstr.tile([64, F], bf16, tag="woc", bufs=2, name="woc")
                dmaq(woc, wo_d[hk * 64 : (hk + 1) * 64, :])
                for rt in (2 * rh, 2 * rh + 1):
                    for nh in range(2):
                        nc.tensor.matmul(
                            px1[rt][:, nh * 512 : (nh + 1) * 512],
                            outT[hk][:, rt * 128 : (rt + 1) * 128],
                            woc[:, nh * 512 : (nh + 1) * 512],
                            start=(hk == 0),
                            stop=(hk == H - 1),
                        )
            for rt in (2 * rh, 2 * rh + 1):
                nc.vector.tensor_add(out=x1[rt], in0=px1[rt], in1=h_res[rt])
                nc.vector.tensor_add(out=x1[rt], in0=x1[rt], in1=wob_bc)

        cm_attnp.__exit__(None, None, None)
        cm_aera.__exit__(None, None, None)
        cm_hera.__exit__(None, None, None)

        # ---------------- phase 6: adaLN-2 + transpose ----------------
        h2T = [
            persist.tile([128, R], bf16, tag=f"hT{ft}", name=f"h2T{ft}")
            for ft in range(FT)
        ]
        for rt in range(RB):
            h2_bf = work.tile([128, F], bf16, tag="hnbf", bufs=1, name="h2_bf")
            layer_norm(x1[rt], bc["Wf_bc"], bc["Bf_bc"], h2_bf)
            transpose_to(h2_bf, h2T, rt)

        # ---------------- phase 7: mlp1 + gelu ----------------
        g1T = [persist.tile([128, R], bf16, name=f"g1T{mt}") for mt in range(MFT)]
        for mt in range(MFT):
            w1blk = wstr.tile([128, FT * 128], bf16, tag="w1b", bufs=3, name="w1blk")
            dmaq(out=w1blk, in_=w1_d[mt])
            pg = psA.tile([128, 512], f32, tag="ps", name="pg")
            for kt in range(FT):
                nc.tensor.matmul(
                    pg,
                    w1blk[:, kt * 128 : (kt + 1) * 128],
                    h2T[kt],
                    start=(kt == 0),
                    stop=(kt == FT - 1),
                )
            nc.scalar.activation(
                out=g1T[mt],
                in_=pg,
                func=AF.Gelu,
                bias=b1_sb[:, mt : mt + 1],
                scale=1.0,
            )

        # ---------------- phase 8: mlp2 + residual -> y ----------------
        # F split in half; 4 row-tile accumulators live in psA; w2 is read
        # exactly once (each half-column sweep reads its half of every chunk).
        for fh in range(2):
            pf = {}
            for rt in range(RB):
                pf[rt] = psA.tile([128, 512], f32, tag="ps", name=f"pf{rt}")
            for kt in range(MFT):
                w2c = wstr.tile([128, 512], bf16, tag="w2c", bufs=3, name="w2c")
                dmaq(w2c, w2_d[kt * 128 : (kt + 1) * 128, fh * 512 : (fh + 1) * 512])
                for rt in range(RB):
                    nc.tensor.matmul(
                        pf[rt],
                        g1T[kt][:, rt * 128 : (rt + 1) * 128],
                        w2c,
                        start=(kt == 0),
                        stop=(kt == MFT - 1),
                    )
            for rt in range(RB):
                sl = slice(fh * 512, (fh + 1) * 512)
                yh = work.tile([128, 512], f32, tag="yh", bufs=2, name="yh")
                nc.vector.tensor_add(out=yh, in0=pf[rt], in1=x1[rt][:, sl])
                nc.vector.tensor_add(out=yh, in0=yh, in1=b2_bc[:, sl])
                nc.sync.dma_start(out=y_d[rt * 128 : (rt + 1) * 128, sl], in_=yh)

    nc.compile()
    return nc


def _prep_in_maps(inputs):
    f32 = np.float32
    wmod = np.concatenate(
        [inputs["amod_gw"], inputs["amod_bw"], inputs["fmod_gw"], inputs["fmod_bw"]],
        axis=1,
    ).astype(BF16)
    modb = np.concatenate(
        [inputs["amod_gb"], inputs["amod_bb"], inputs["fmod_gb"], inputs["fmod_bb"]]
    ).astype(f32)
    lnvec = np.stack(
        [
            inputs["amod_nw"],
            inputs["amod_nb"],
            inputs["attn_nw"],
            inputs["attn_nb"],
            inputs["fmod_nw"],
            inputs["fmod_nb"],
        ]
    ).astype(f32)
    wq_t = np.ascontiguousarray(
        np.asarray(inputs["wq"]).astype(BF16).reshape(FT, 128, MT, 128)
        .transpose(2, 1, 0, 3).reshape(MT, 128, FT * 128)
    )
    w1_t = np.ascontiguousarray(
        np.asarray(inputs["w1"]).astype(BF16).reshape(FT, 128, MFT, 128)
        .transpose(2, 1, 0, 3).reshape(MFT, 128, FT * 128)
    )
    shared = dict(
        wmod=wmod,
        modb=modb,
        lnvec=lnvec,
        wq=wq_t,
        wkv=np.asarray(inputs["wkv"]).astype(BF16),
        wo=np.asarray(inputs["wo"]).astype(BF16),
        wo_bias=np.asarray(inputs["wo_b"]).astype(f32).reshape(1, F),
        w1=w1_t,
        b1=np.asarray(inputs["b1"]).astype(f32),
        w2=np.asarray(inputs["w2"]).astype(BF16),
        b2=np.asarray(inputs["b2"]).astype(f32).reshape(1, F),
    )
    x = np.asarray(inputs["x"]).astype(f32)
    cond = np.asarray(inputs["cond"]).astype(BF16)
    in_maps = []
    for c in range(NCORES):
        b, r0 = c // 4, (c % 4) * R
        m = dict(shared)
        m["x"] = np.ascontiguousarray(x[b, r0 : r0 + R, :])
        m["cond"] = np.ascontiguousarray(cond[b])
        in_maps.append(m)
    return in_maps


def _run(inputs, trace=False):
    from concourse.bass_utils import run_bass_kernel_spmd

    if "nc" not in _CACHE:
        _CACHE["nc"] = _build_nc()
    nc = _CACHE["nc"]
    in_maps = _prep_in_maps(inputs)
    res = run_bass_kernel_spmd(
        nc, in_maps, core_ids=list(range(NCORES)), trace=trace
    )
    y = np.empty((B, T, F), np.float32)
    for c in range(NCORES):
        b, r0 = c // 4, (c % 4) * R
        y[b, r0 : r0 + R, :] = res.results[c]["y"]
    return y, res


def kernel(**inputs) -> np.ndarray:
    y, _ = _run(inputs, trace=False)
    return y


if __name__ == "__main__":
    _build_nc()
    print("build OK")



# revision 22
# speedup vs baseline: 1.2402x; 1.2402x over previous
"""DiT block kernel for 8x Trainium2 NeuronCores (Bass/Tile).

Sharding: row-parallel over the flattened (B,T)=4096 rows; 512 rows/core.
Cores 0-3 handle batch 0, cores 4-7 batch 1. MQA K/V is computed per-shard
and AllGather'd within each 4-core batch group. Weights are replicated and
cast to bf16; LN/residual math stays fp32.

v2 optimizations over the first working version:
  - MM1 (QK^T, K=64) and the out-projection (K=64) are row-tiled: even
    heads occupy PE rows 0-63, odd heads rows 64-127, with issue order
    interleaved so pairs run concurrently on the PE array (~2x).
  - exp runs on [128,1024] PSUM spans (two matmul banks per activation)
    to amortize the ~352-cycle ACT instruction overhead; a 3-slot PSUM
    rotation keeps exp back-to-back (it is the attention bottleneck).
  - the gpsimd queue holds only the adaLN adds + the AllGather so the
    collective issues early.
  - wq/wo/w1 prefetch into SBUF on the sync/scalar HWDGE queues before
    attention (DMA was idle there); w2 streams during mlp1/mlp2.
  - wo_b and b2 biases fold into the matmul accumulation via a ones-row
    matmul instead of DVE adds.
  - dummy warmup/filler matmuls keep the PE HAM clock-gate at 8/8.
  - SBUF pools are split by live range over the left/right allocation
    stacks (pools charge their footprint at open).
"""

import os
import sys

sys.path.insert(0, "/opt/trn_rl_repo")

import numpy as np
import ml_dtypes

BF16 = ml_dtypes.bfloat16

B, T, F, H, D, M, C = 2, 2048, 1024, 16, 64, 4, 512
NCORES = 8
R = (B * T) // NCORES  # 512 rows per core
RB = R // 128  # 4 row blocks
FT = F // 128  # 8 feature tiles
MT = (H * D) // 128  # 8 head-pair tiles
MFT = (M * F) // 128  # 32 mlp hidden tiles
KT = T // 128  # 16 key tiles
EPS = 1e-5

_CACHE = {}


def _build_nc():
    import concourse.bass as bass
    import concourse.tile as tile
    from concourse import bacc, mybir
    from concourse.masks import make_identity
    from contextlib import ExitStack

    f32 = mybir.dt.float32
    f16 = mybir.dt.float16
    bf16 = mybir.dt.bfloat16
    AF = mybir.ActivationFunctionType
    OP = mybir.AluOpType

    nc = bacc.Bacc(
        "TRN2",
        target_bir_lowering=False,
        debug=False,
        enable_asserts=False,
        num_devices=NCORES,
    )

    def dram(name, shape, dt, **kw):
        return nc.dram_tensor(name, shape, dt, **kw).ap()

    x_d = dram("x", [R, F], f32, kind="ExternalInput")
    cond_d = dram("cond", [C], bf16, kind="ExternalInput")
    wmod_d = dram("wmod", [C, 4 * F], bf16, kind="ExternalInput")
    modb_d = dram("modb", [4 * F], f32, kind="ExternalInput")
    lnv_d = dram("lnvec", [6, F], f32, kind="ExternalInput")
    lnv16_d = dram("lnvec16", [6, F], bf16, kind="ExternalInput")
    wq_d = dram("wq", [MT, 128, FT * 128], bf16, kind="ExternalInput")
    wkv_d = dram("wkv", [F, 2 * D], bf16, kind="ExternalInput")
    # wo pre-paired: [pair, 128(d of even head | d of odd head), F]
    wo_d = dram("wo", [MT, 128, F], bf16, kind="ExternalInput")
    wob_d = dram("wo_bias", [1, F], bf16, kind="ExternalInput")
    w1_d = dram("w1", [MFT, 128, FT * 128], bf16, kind="ExternalInput")
    b1_d = dram("b1", [M * F], f32, kind="ExternalInput")
    w2_d = dram("w2", [M * F, F], bf16, kind="ExternalInput")
    b2_d = dram("b2", [1, F], bf16, kind="ExternalInput")
    y_d = dram("y", [R, F], f32, kind="ExternalOutput")

    groups = [[0, 1, 2, 3], [4, 5, 6, 7]]

    def bcast_row(ap_row):
        # [1, n] DRAM AP -> partition-broadcast [128, n]
        return bass.AP(
            tensor=ap_row.tensor,
            offset=ap_row.offset,
            ap=[[0, 128]] + list(ap_row.ap[-1:]),
        )

    with tile.TileContext(nc) as tc, ExitStack() as ctx:
        # left stack: consts, work, hTp, w1p, hera, aera, wop, [attnp]
        # right stack: wqp, xp, [modtmp], then x1p, g1p
        consts = ctx.enter_context(tc.tile_pool(name="consts", bufs=1))
        work = ctx.enter_context(tc.tile_pool(name="work", bufs=2))
        cm_hTp = tc.tile_pool(name="hTp", bufs=1)
        hTp = cm_hTp.__enter__()
        cm_wqp = tc.tile_pool(name="wqp", bufs=1, side="right")
        wqp = cm_wqp.__enter__()
        cm_xp = tc.tile_pool(name="xp", bufs=1, side="right")
        xp = cm_xp.__enter__()
        dramp = ctx.enter_context(tc.tile_pool(name="dramp", bufs=1, space="DRAM"))

        # ---------------- constants (sync queue DMAs) ----------------
        ident = consts.tile([128, 128], bf16, name="ident")
        make_identity(nc, ident)
        ones16 = consts.tile([1, 128], f16, name="ones16")
        nc.vector.memset(ones16, 1.0)
        onescol = consts.tile([1, 128], bf16, name="onescol")
        nc.vector.memset(onescol, 1.0)
        epst = consts.tile([128, 1], f32, name="epst")
        nc.vector.memset(epst, EPS)
        # ones rows at partitions 64 (even-head denom) and 32 (odd-head denom)
        ones2 = consts.tile([128, 64], f16, name="ones2")
        nc.vector.memset(ones2[64:65, :], 1.0)
        nc.vector.memset(ones2[32:33, :], 1.0)

        cond_sb = consts.tile([128, 4], bf16, name="cond_sb")
        nc.sync.dma_start(out=cond_sb, in_=cond_d.rearrange("(a p) -> p a", p=128))
        b1_sb = consts.tile([128, MFT], f32, name="b1_sb")
        nc.sync.dma_start(out=b1_sb, in_=b1_d.rearrange("(mt p) -> p mt", p=128))
        wob_sb = consts.tile([1, F], bf16, name="wob_sb")
        nc.sync.dma_start(out=wob_sb, in_=wob_d)
        b2_sb = consts.tile([1, F], bf16, name="b2_sb")
        nc.sync.dma_start(out=b2_sb, in_=b2_d)
        wkv_sb = consts.tile([128, FT, 2 * D], bf16, name="wkv_sb")
        nc.sync.dma_start(
            out=wkv_sb, in_=wkv_d.rearrange("(kt p) n -> p kt n", p=128)
        )
        anw_bc = consts.tile([128, F], f32, name="anw_bc")
        nc.sync.dma_start(out=anw_bc, in_=bcast_row(lnv_d[2:3, :]))
        anb_bc = consts.tile([128, F], f32, name="anb_bc")
        nc.sync.dma_start(out=anb_bc, in_=bcast_row(lnv_d[3:4, :]))

        # x row blocks (sync queue, ahead of wq: needed first)
        xs = []
        for rb in range(RB):
            x_rb = xp.tile([128, F], f32, name=f"x{rb}")
            nc.sync.dma_start(out=x_rb, in_=x_d[rb * 128 : (rb + 1) * 128, :])
            xs.append(x_rb)

        # wq full prefetch (sync queue; needed from ~phase 2)
        wq_sb = wqp.tile([128, MT, FT * 128], bf16, name="wq_sb")
        for mt in range(MT):
            nc.sync.dma_start(out=wq_sb[:, mt, :], in_=wq_d[mt])

        # ---------------- phase 0: modulation vectors ----------------
        cm_modtmp = tc.tile_pool(name="modtmp", bufs=1, side="right")
        modtmp = cm_modtmp.__enter__()

        def filler(pool, n):
            # dummy matmuls: keep the PE HAM activity window busy
            for _ in range(n):
                wps = pool.tile([128, 512], f32, tag="warm", name="warm")
                nc.tensor.matmul(wps[:, 0:128], ident, ident, start=True, stop=True)

        lnr = {}
        for r in (0, 1, 4, 5):  # amod_nw/nb, fmod_nw/nb rows at partition 0
            lnr[r] = modtmp.tile([1, F], bf16, name=f"lnr{r}")
            nc.sync.dma_start(out=lnr[r], in_=lnv16_d[r : r + 1, :])
        modb_sb = modtmp.tile([1, 4 * F], f32, name="modb_sb")
        nc.sync.dma_start(out=modb_sb, in_=modb_d.rearrange("(a f) -> a f", a=1))
        modv = modtmp.tile([1, 4 * F], f32, name="modv")
        tmpv = modtmp.tile([1, F], f32, name="tmpv")
        modv16 = modtmp.tile([1, 4 * F], f16, name="modv16")
        bc = {}

        def mod_matmuls(ps_pool, grp):
            wm_tiles = []
            for ch in range(4):
                wm = modtmp.tile(
                    [128, 2048], bf16, tag="wm", bufs=2, name=f"wm{grp}_{ch}"
                )
                nc.scalar.dma_start(
                    out=wm,
                    in_=wmod_d[
                        ch * 128 : (ch + 1) * 128, grp * 2048 : (grp + 1) * 2048
                    ],
                )
                wm_tiles.append(wm)
            pms = [
                ps_pool.tile([128, 512], f32, tag=f"pm{j}", name=f"pm{j}")
                for j in range(4)
            ]
            for ch in range(4):
                for j in range(4):
                    nc.tensor.matmul(
                        pms[j][0:1, :],
                        cond_sb[:, ch : ch + 1],
                        wm_tiles[ch][:, j * 512 : (j + 1) * 512],
                        start=(ch == 0),
                        stop=(ch == 3),
                    )
            for j in range(4):
                nb = grp * 4 + j
                nc.vector.tensor_add(
                    out=modv[:, nb * 512 : (nb + 1) * 512],
                    in0=pms[j][0:1, :],
                    in1=modb_sb[:, nb * 512 : (nb + 1) * 512],
                )

        def finalize_mod(ps_pool, g_off, b_off, nw_row, nb_row, w_name, b_name):
            g_sl = modv[:, g_off : g_off + F]
            b_sl = modv[:, b_off : b_off + F]
            nc.scalar.add(out=g_sl, in_=g_sl, add=1.0)
            nc.vector.tensor_mul(out=tmpv, in0=g_sl, in1=lnr[nb_row])
            with nc.allow_low_precision(reason="f16 staging for PE broadcast"):
                nc.vector.tensor_add(
                    out=modv16[:, b_off : b_off + F], in0=tmpv, in1=b_sl
                )
                nc.vector.tensor_mul(
                    out=modv16[:, g_off : g_off + F], in0=g_sl, in1=lnr[nw_row]
                )
            for off, nm in ((g_off, w_name), (b_off, b_name)):
                bt = consts.tile([128, F], bf16, name=nm)
                for hf in range(2):
                    pb = ps_pool.tile([128, 512], f32, tag="pbc", bufs=2, name="pbc")
                    nc.tensor.matmul(
                        pb,
                        ones16,
                        modv16[:, off + hf * 512 : off + (hf + 1) * 512],
                        start=True,
                        stop=True,
                    )
                    nc.scalar.activation(
                        bt[:, hf * 512 : (hf + 1) * 512], pb, AF.Copy
                    )
                bc[nm] = bt

        with tc.tile_pool(name="ps0", bufs=1, space="PSUM") as ps0:
            filler(ps0, 40)
            mod_matmuls(ps0, 0)
            finalize_mod(ps0, 0, F, 0, 1, "Wa_bc", "Ba_bc")
            mod_matmuls(ps0, 1)
            filler(ps0, 20)
            finalize_mod(ps0, 2 * F, 3 * F, 4, 5, "Wf_bc", "Bf_bc")
        cm_modtmp.__exit__(None, None, None)
        cm_w1p = tc.tile_pool(name="w1p", bufs=1)
        w1p = cm_w1p.__enter__()
        cm_hera = tc.tile_pool(name="hera", bufs=1)
        hera = cm_hera.__enter__()
        cm_aera = tc.tile_pool(name="aera", bufs=1)
        aera = cm_aera.__enter__()
        cm_wop = tc.tile_pool(name="wop", bufs=1)
        wop = cm_wop.__enter__()

        # ---------------- helpers ----------------
        def layer_norm(src, w_bc, b_bc, out_tile, badd_engine):
            """out = LN(src) * w_bc + b_bc ; src [128,F] f32."""
            stats = work.tile([128, 2, 6], f32, tag="stats", name="stats")
            for sg in range(2):
                nc.vector.bn_stats(
                    out=stats[:, sg, :], in_=src[:, sg * 512 : (sg + 1) * 512]
                )
            mv = work.tile([128, 2], f32, tag="mv", name="mv")
            nc.vector.bn_aggr(out=mv, in_=stats)
            rstd = work.tile([128, 1], f32, tag="rstd", name="rstd")
            nc.scalar.activation(
                out=rstd, in_=mv[:, 1:2], func=AF.Sqrt, bias=epst, scale=1.0
            )
            nc.vector.reciprocal(out=rstd, in_=rstd)
            xn = work.tile([128, F], f32, tag="xn", name="xn")
            nc.vector.tensor_scalar(
                out=xn,
                in0=src,
                scalar1=mv[:, 0:1],
                scalar2=rstd,
                op0=OP.subtract,
                op1=OP.mult,
            )
            nc.vector.tensor_mul(out=xn, in0=xn, in1=w_bc)
            badd_engine.tensor_add(out=out_tile, in0=xn, in1=b_bc)

        def transpose_to(ps_pool, bufs, hsrc_bf, hT_tiles, rb):
            """hsrc_bf [128,F] bf16 -> hT_tiles[ft][:, rb*128:+128]."""
            for ft in range(FT):
                pt = ps_pool.tile([128, 128], bf16, tag="ptt", bufs=bufs, name="ptt")
                nc.tensor.transpose(
                    pt, hsrc_bf[:, ft * 128 : (ft + 1) * 128], ident
                )
                nc.scalar.activation(
                    out=hT_tiles[ft][:, rb * 128 : (rb + 1) * 128],
                    in_=pt,
                    func=AF.Copy,
                )

        hT = [
            hTp.tile([128, R], bf16, tag=f"hT{ft}", name=f"hT{ft}")
            for ft in range(FT)
        ]

        cm_ps1 = tc.tile_pool(name="ps1", bufs=1, space="PSUM")
        ps1 = cm_ps1.__enter__()

        # ---------------- phase 1: adaLN-1 + attn-LN + transpose ----------------
        h_res = [hera.tile([128, F], f32, name=f"h{rb}") for rb in range(RB)]
        for rb in range(RB):
            # adaLN add on gpsimd: done before the collective reaches the queue
            layer_norm(xs[rb], bc["Wa_bc"], bc["Ba_bc"], h_res[rb], nc.gpsimd)
            hn_bf = work.tile([128, F], bf16, tag="hnbf", bufs=2, name="hn_bf")
            layer_norm(h_res[rb], anw_bc, anb_bc, hn_bf, nc.vector)
            transpose_to(ps1, 4, hn_bf, hT, rb)
        cm_xp.__exit__(None, None, None)

        # ---------------- phase 2: kv first (AllGather ASAP), then q ----------------
        pkv = ps1.tile([128, 512], f32, tag="pkq", bufs=2, name="pkv")
        for kt in range(FT):
            nc.tensor.matmul(
                pkv, wkv_sb[:, kt, :], hT[kt], start=(kt == 0), stop=(kt == FT - 1)
            )
        kvT_sb = work.tile([128, R], bf16, tag="kvT", bufs=1, name="kvT_sb")
        nc.scalar.activation(out=kvT_sb, in_=pkv, func=AF.Copy)

        kvT_bounce = dramp.tile([2 * D, R], bf16, name="kvT_bounce")
        kvT_all = dramp.tile([4 * 2 * D, R], bf16, name="kvT_all")
        nc.sync.dma_start(out=kvT_bounce, in_=kvT_sb)
        nc.gpsimd.collective_compute(
            "AllGather",
            OP.bypass,
            replica_groups=groups,
            ins=[kvT_bounce[:, :]],
            outs=[kvT_all[:, :]],
        )

        # q projection fills the AllGather wait
        qT = [aera.tile([128, R], bf16, tag=f"qo{mt}", name=f"qT{mt}") for mt in range(MT)]
        for mt in range(MT):
            pq = ps1.tile([128, 512], f32, tag="pkq", bufs=2, name="pq")
            for kt in range(FT):
                nc.tensor.matmul(
                    pq,
                    wq_sb[:, mt, kt * 128 : (kt + 1) * 128],
                    hT[kt],
                    start=(kt == 0),
                    stop=(kt == FT - 1),
                )
            # fold the attention 1/sqrt(D)=0.125 scale into q
            nc.scalar.activation(out=qT[mt], in_=pq, func=AF.Copy, scale=0.125)

        # weight prefetch dispatched before attention so transfers run
        # under it: wo + w1 first half (scalar q), w1 rest (sync q, below)
        wo_sb = wop.tile([128, MT, F], bf16, name="wo_sb")
        for mt in range(MT):
            nc.scalar.dma_start(out=wo_sb[:, mt, :], in_=wo_d[mt])
        W1PRE = 16
        W1MID = 8
        w1_sb = w1p.tile([128, W1PRE, FT * 128], bf16, name="w1_sb")
        for mt in range(W1PRE):
            nc.scalar.dma_start(out=w1_sb[:, mt, :], in_=w1_d[mt])
        w1b_sb = w1p.tile([128, W1MID, FT * 128], bf16, name="w1b_sb")

        # ---------------- phase 3: kT / v_ext assembly ----------------
        kT = aera.tile([128, T], bf16, name="kT")
        for hp in (0, 64):
            for r in range(4):
                nc.sync.dma_start(
                    out=kT[hp : hp + 64, r * R : (r + 1) * R],
                    in_=kvT_all[r * 128 : r * 128 + 64, :],
                )
        v_e = [aera.tile([128, 65], bf16, name=f"ve{kt}") for kt in range(KT)]
        v_o = [aera.tile([128, 128], bf16, name=f"vo{kt}") for kt in range(KT)]
        for kt in range(KT):
            nc.vector.memset(v_e[kt][:, 64:65], 1.0)
            nc.vector.memset(v_o[kt], 0.0)
            nc.vector.memset(v_o[kt][:, 32:33], 1.0)
        filler(ps1, 30)
        for r in range(4):
            vT_sb = work.tile([64, R], bf16, tag="vTs", bufs=2, name="vT_sb")
            nc.sync.dma_start(
                out=vT_sb, in_=kvT_all[r * 128 + 64 : (r + 1) * 128, :]
            )
            for cc in range(4):
                ptv = ps1.tile([128, 128], bf16, tag="ptt", bufs=4, name="ptv")
                nc.tensor.matmul(
                    ptv[:, 0:64],
                    vT_sb[:, cc * 128 : (cc + 1) * 128],
                    ident[0:64, 0:64],
                    is_transpose=True,
                )
                kt = r * 4 + cc
                nc.vector.tensor_copy(out=v_e[kt][:, 0:64], in_=ptv[:, 0:64])
                nc.vector.tensor_copy(out=v_o[kt][:, 64:128], in_=ptv[:, 0:64])

        # w1 middle chunk on the sync queue (idle from here to phase 8):
        # transfers run during attention; the last 8 chunks ring-stream.
        for mt in range(W1MID):
            nc.sync.dma_start(out=w1b_sb[:, mt, :], in_=w1_d[W1PRE + mt])

        cm_wqp.__exit__(None, None, None)
        cm_ps1.__exit__(None, None, None)

        # ---------------- phase 4: attention ----------------
        # transposed scores [keys, rows]; heads paired (even at PE rows
        # 0-63, odd at rows 64-127) so MM1 row-tiles 2x. exp covers
        # [128,1024] (two kt) per ACT instruction; 3 rotating 2-bank PSUM
        # slots keep exp back-to-back (scalar engine is the bottleneck).
        cm_ps4 = tc.tile_pool(name="ps4", bufs=1, space="PSUM")
        ps4 = cm_ps4.__enter__()
        cm_attnp = tc.tile_pool(name="attnp", bufs=1)
        attnp = cm_attnp.__enter__()

        outT = [aera.tile([128, R], bf16, tag=f"qo{mt}", name=f"outT{mt}") for mt in range(MT)]

        STOP = int(os.environ.get("STOP_AFTER", "99"))
        for mt in range(MT if STOP >= 4 else 0):
            po_e = ps4.tile([128, 512], f32, tag="po", bufs=2, name="po_e")
            po_o = ps4.tile([128, 512], f32, tag="po", bufs=2, name="po_o")
            EXPW = 1 if os.environ.get("EXP_NARROW") else 2
            for ktt in range(16 // EXPW):
                kt0 = EXPW * ktt
                ps_e = ps4.tile(
                    [128, 512 * EXPW], f32, tag="mm1", bufs=6 // EXPW, name="ps_e"
                )
                ps_o = ps4.tile(
                    [128, 512 * EXPW], f32, tag="mm1", bufs=6 // EXPW, name="ps_o"
                )
                for i in range(EXPW):
                    ksl = kT[:, (kt0 + i) * 128 : (kt0 + i + 1) * 128]
                    nc.tensor.matmul(
                        ps_e[:, i * 512 : (i + 1) * 512],
                        ksl[0:64, :],
                        qT[mt][0:64, :],
                        start=True,
                        stop=True,
                    )
                    nc.tensor.matmul(
                        ps_o[:, i * 512 : (i + 1) * 512],
                        ksl[64:128, :],
                        qT[mt][64:128, :],
                        start=True,
                        stop=True,
                    )
                pr_e = attnp.tile(
                    [128, 512 * EXPW], bf16, tag="pr", bufs=6 // EXPW, name="pr_e"
                )
                pr_o = attnp.tile(
                    [128, 512 * EXPW], bf16, tag="pr", bufs=6 // EXPW, name="pr_o"
                )
                nc.scalar.activation(out=pr_e, in_=ps_e, func=AF.Exp)
                nc.scalar.activation(out=pr_o, in_=ps_o, func=AF.Exp)
                for i in range(EXPW):
                    kt = kt0 + i
                    nc.tensor.matmul(
                        po_e[0:65, :],
                        v_e[kt][:, 0:65],
                        pr_e[:, i * 512 : (i + 1) * 512],
                        start=(kt == 0),
                        stop=(kt == KT - 1),
                    )
                    nc.tensor.matmul(
                        po_o,
                        v_o[kt],
                        pr_o[:, i * 512 : (i + 1) * 512],
                        start=(kt == 0),
                        stop=(kt == KT - 1),
                    )
            # denominators: even head at po_e[64], odd at po_o[32];
            # broadcast reciprocals over partitions via tiny K=1 matmuls.
            rcp = work.tile([128, R], f16, tag="rcp", bufs=2, name="rcp")
            with nc.allow_low_precision(reason="f16 softmax denom broadcast"):
                nc.vector.reciprocal(out=rcp[64:65, :], in_=po_e[64:65, :])
                nc.vector.reciprocal(out=rcp[32:33, :], in_=po_o[32:33, :])
            bcr = ps4.tile(
                [128, 512 * EXPW], f32, tag="mm1", bufs=6 // EXPW, name="bcr"
            )
            nc.tensor.matmul(
                bcr[0:64, 0:512], ones2[64:65, :], rcp[64:65, :],
                start=True, stop=True,
            )
            nc.tensor.matmul(
                bcr[64:128, 0:512], ones2[32:33, :], rcp[32:33, :],
                start=True, stop=True,
            )
            t_sb = work.tile([128, R], bf16, tag="tsb", bufs=2, name="t_sb")
            nc.vector.tensor_copy(out=t_sb[0:64, :], in_=po_e[0:64, :])
            nc.vector.tensor_copy(out=t_sb[64:128, :], in_=po_o[64:128, :])
            nc.vector.tensor_mul(
                out=outT[mt][0:64, :], in0=t_sb[0:64, :], in1=bcr[0:64, 0:512]
            )
            nc.vector.tensor_mul(
                out=outT[mt][64:128, :], in0=t_sb[64:128, :], in1=bcr[64:128, 0:512]
            )

        cm_attnp.__exit__(None, None, None)
        cm_ps4.__exit__(None, None, None)

        # ---------------- phase 5+6: out proj -> x1 -> adaLN-2 ----------------
        cm_x1p = tc.tile_pool(name="x1p", bufs=1, side="right")
        x1p = cm_x1p.__enter__()
        cm_ps56 = tc.tile_pool(name="ps56", bufs=1, space="PSUM")
        ps56 = cm_ps56.__enter__()

        x1 = [x1p.tile([128, F], f32, name=f"x1_{rt}") for rt in range(RB)]
        h2T = [
            hTp.tile([128, R], bf16, tag=f"hT{ft}", name=f"h2T{ft}")
            for ft in range(FT)
        ]
        # even and odd heads accumulate into SEPARATE psum tiles (two
        # concurrent PE row-tiles must not write the same psum addresses);
        # the DVE merges them into x1.
        for rt in range(RB if STOP >= 5 else 0):
            px_e = ps56.tile([128, F], f32, tag="pxe", bufs=1, name="px_e")
            px_o = ps56.tile([128, F], f32, tag="pxo", bufs=1, name="px_o")
            rsl = slice(rt * 128, (rt + 1) * 128)
            for mt in range(MT):
                for nh in range(2):
                    fsl = slice(nh * 512, (nh + 1) * 512)
                    nc.tensor.matmul(
                        px_e[:, fsl],
                        outT[mt][0:64, rsl],
                        wo_sb[0:64, mt, fsl],
                        start=(mt == 0),
                        stop=False,
                    )
                    nc.tensor.matmul(
                        px_o[:, fsl],
                        outT[mt][64:128, rsl],
                        wo_sb[64:128, mt, fsl],
                        start=(mt == 0),
                        stop=(mt == MT - 1),
                    )
            # wo bias via ones-row matmul closes the even accumulation
            for nh in range(2):
                fsl = slice(nh * 512, (nh + 1) * 512)
                nc.tensor.matmul(
                    px_e[:, fsl], onescol, wob_sb[:, fsl],
                    start=False, stop=True,
                )
            nc.vector.tensor_add(out=x1[rt], in0=px_e, in1=h_res[rt])
            nc.vector.tensor_add(out=x1[rt], in0=x1[rt], in1=px_o)
            if STOP < 6:
                continue
            h2_bf = work.tile([128, F], bf16, tag="hnbf", bufs=2, name="h2_bf")
            layer_norm(x1[rt], bc["Wf_bc"], bc["Bf_bc"], h2_bf, nc.vector)
            transpose_to(ps56, 2, h2_bf, h2T, rt)

        cm_ps56.__exit__(None, None, None)
        cm_wop.__exit__(None, None, None)
        cm_aera.__exit__(None, None, None)
        cm_hera.__exit__(None, None, None)

        # ---------------- phase 7: mlp1 + gelu ----------------
        cm_ps78 = tc.tile_pool(name="ps78", bufs=1, space="PSUM")
        ps78 = cm_ps78.__enter__()

        w1tail = {}
        for mt in range(W1PRE + W1MID, MFT if STOP >= 7 else 0):
            t = work.tile([128, FT * 128], bf16, tag="w1c", bufs=4, name="w1c")
            nc.sync.dma_start(out=t, in_=w1_d[mt])
            w1tail[mt] = t

        # w2 even chunks stream on the sync queue (no compute there, so
        # ring-slot waits cannot deadlock); odd chunks dispatch on the
        # scalar queue AFTER the gelus (a dispatch before them would wait
        # on phase-8 matmuls that wait on the gelus -> queue deadlock).
        w2c = {}
        for fh in range(2 if STOP >= 8 else 0):
            for kt in range(0, MFT, 2):
                t = work.tile([128, 512], bf16, tag="w2cs", bufs=4, name="w2cs")
                nc.sync.dma_start(
                    out=t,
                    in_=w2_d[kt * 128 : (kt + 1) * 128, fh * 512 : (fh + 1) * 512],
                )
                w2c[(fh, kt)] = t

        cm_g1p = tc.tile_pool(name="g1p", bufs=1, side="right")
        g1p = cm_g1p.__enter__()
        g1T = [g1p.tile([128, R], bf16, name=f"g1T{mt}") for mt in range(MFT)]
        for mt in range(MFT if STOP >= 7 else 0):
            wsrc = (
                w1_sb[:, mt, :] if mt < W1PRE
                else w1b_sb[:, mt - W1PRE, :] if mt < W1PRE + W1MID
                else w1tail[mt]
            )
            pg = ps78.tile([128, 512], f32, tag="pg", bufs=4, name="pg")
            for kt in range(FT):
                nc.tensor.matmul(
                    pg,
                    wsrc[:, kt * 128 : (kt + 1) * 128],
                    h2T[kt],
                    start=(kt == 0),
                    stop=(kt == FT - 1),
                )
            if os.environ.get("SIM_SAFE"):
                nc.scalar.activation(out=g1T[mt], in_=pg, func=AF.Exp)
            else:
                nc.scalar.activation(
                    out=g1T[mt], in_=pg, func=AF.Gelu,
                    bias=b1_sb[:, mt : mt + 1], scale=1.0,
                )

        for fh in range(2 if STOP >= 8 else 0):
            for kt in range(1, MFT, 2):
                t = work.tile([128, 512], bf16, tag="w2ca", bufs=4, name="w2ca")
                nc.scalar.dma_start(
                    out=t,
                    in_=w2_d[kt * 128 : (kt + 1) * 128, fh * 512 : (fh + 1) * 512],
                )
                w2c[(fh, kt)] = t

        # ---------------- phase 8: mlp2 + residual -> y ----------------
        if STOP < 8:
            for rt in range(RB):
                yh = work.tile([128, F], f32, tag="ydummy", bufs=2, name="ydummy")
                nc.vector.memset(yh, 0.0)
                nc.sync.dma_start(out=y_d[rt * 128 : (rt + 1) * 128, :], in_=yh)
        for fh in range(2 if STOP >= 8 else 0):
            pf = {}
            for rt in range(RB):
                pf[rt] = ps78.tile([128, 512], f32, tag="pg", bufs=4, name=f"pf{rt}")
            for kt in range(MFT):
                for rt in range(RB):
                    nc.tensor.matmul(
                        pf[rt],
                        g1T[kt][:, rt * 128 : (rt + 1) * 128],
                        w2c[(fh, kt)],
                        start=(kt == 0),
                        stop=False,
                    )
            fsl = slice(fh * 512, (fh + 1) * 512)
            for rt in range(RB):
                nc.tensor.matmul(
                    pf[rt], onescol, b2_sb[:, fsl], start=False, stop=True
                )
            for rt in range(RB):
                yh = work.tile([128, 512], f32, tag="yh", bufs=4, name="yh")
                nc.vector.tensor_add(out=yh, in0=pf[rt], in1=x1[rt][:, fsl])
                nc.sync.dma_start(out=y_d[rt * 128 : (rt + 1) * 128, fsl], in_=yh)

        cm_g1p.__exit__(None, None, None)
        cm_x1p.__exit__(None, None, None)
        cm_ps78.__exit__(None, None, None)
        cm_w1p.__exit__(None, None, None)
        cm_hTp.__exit__(None, None, None)

    nc.compile()
    return nc


def _prep_in_maps(inputs):
    f32 = np.float32
    wmod = np.concatenate(
        [inputs["amod_gw"], inputs["amod_bw"], inputs["fmod_gw"], inputs["fmod_bw"]],
        axis=1,
    ).astype(BF16)
    modb = np.concatenate(
        [inputs["amod_gb"], inputs["amod_bb"], inputs["fmod_gb"], inputs["fmod_bb"]]
    ).astype(f32)
    lnvec = np.stack(
        [
            inputs["amod_nw"],
            inputs["amod_nb"],
            inputs["attn_nw"],
            inputs["attn_nb"],
            inputs["fmod_nw"],
            inputs["fmod_nb"],
        ]
    ).astype(f32)
    wq_t = np.ascontiguousarray(
        np.asarray(inputs["wq"]).astype(BF16).reshape(FT, 128, MT, 128)
        .transpose(2, 1, 0, 3).reshape(MT, 128, FT * 128)
    )
    w1_t = np.ascontiguousarray(
        np.asarray(inputs["w1"]).astype(BF16).reshape(FT, 128, MFT, 128)
        .transpose(2, 1, 0, 3).reshape(MFT, 128, FT * 128)
    )
    # wo [H*D, F] -> pair layout [MT, 128, F]: partitions 0-63 = head 2i,
    # 64-127 = head 2i+1.
    wo = np.asarray(inputs["wo"]).astype(BF16).reshape(H, D, F)
    wo_t = np.ascontiguousarray(
        np.stack([np.concatenate([wo[2 * i], wo[2 * i + 1]], 0) for i in range(MT)])
    )
    shared = dict(
        wmod=wmod,
        modb=modb,
        lnvec=lnvec,
        lnvec16=lnvec.astype(BF16),
        wq=wq_t,
        wkv=np.asarray(inputs["wkv"]).astype(BF16),
        wo=wo_t,
        wo_bias=np.asarray(inputs["wo_b"]).astype(BF16).reshape(1, F),
        w1=w1_t,
        b1=np.asarray(inputs["b1"]).astype(f32),
        w2=np.asarray(inputs["w2"]).astype(BF16),
        b2=np.asarray(inputs["b2"]).astype(BF16).reshape(1, F),
    )
    x = np.asarray(inputs["x"]).astype(f32)
    cond = np.asarray(inputs["cond"]).astype(BF16)
    in_maps = []
    for c in range(NCORES):
        b, r0 = c // 4, (c % 4) * R
        m = dict(shared)
        m["x"] = np.ascontiguousarray(x[b, r0 : r0 + R, :])
        m["cond"] = np.ascontiguousarray(cond[b])
        in_maps.append(m)
    return in_maps


def _run(inputs, trace=False):
    from concourse.bass_utils import run_bass_kernel_spmd

    if "nc" not in _CACHE:
        _CACHE["nc"] = _build_nc()
    nc = _CACHE["nc"]
    in_maps = _prep_in_maps(inputs)
    res = run_bass_kernel_spmd(
        nc, in_maps, core_ids=list(range(NCORES)), trace=trace
    )
    y = np.empty((B, T, F), np.float32)
    for c in range(NCORES):
        b, r0 = c // 4, (c % 4) * R
        y[b, r0 : r0 + R, :] = res.results[c]["y"]
    return y, res


def kernel(**inputs) -> np.ndarray:
    y, _ = _run(inputs, trace=False)
    return y


if __name__ == "__main__":
    _build_nc()
    print("build OK")


# revision 27
# speedup vs baseline: 1.3708x; 1.1053x over previous
"""DiT block kernel for 8x Trainium2 NeuronCores (Bass/Tile).

Sharding: row-parallel over the flattened (B,T)=4096 rows; 512 rows/core.
Cores 0-3 handle batch 0, cores 4-7 batch 1. MQA K/V is computed per-shard
and AllGather'd within each 4-core batch group. Weights are replicated and
cast to bf16; LN/residual math stays fp32.

v3 notes (on top of v2's row-tiled MM1/outproj, 2-bank exp, prefetch):
  - attention is software-pipelined: MM1 of quad k+1 issues before PV of
    quad k so the scalar-engine exp stream never starves; the per-pair
    softmax tail (denominator copies -> one batched reciprocal -> bcr
    broadcast matmul -> output muls) is deferred into the next pair so
    the slow DVE reciprocal never stalls the PE FIFO.
  - attn-LN's gamma/beta are folded into wq/wkv host-side (bias lands
    via the per-partition bias operand of the qT/kvT PSUM-copy), so the
    second LN is just normalize (saves 2 DVE passes/row-block + 8KB).
  - kvT bounce rides the gpsimd queue right before the AllGather, so the
    collective no longer waits behind megabytes of weight DMA.
  - DMA queues balanced so x/wmod/wq all land before their consumers.
"""

import os
import sys

sys.path.insert(0, "/opt/trn_rl_repo")

import numpy as np
import ml_dtypes

BF16 = ml_dtypes.bfloat16

B, T, F, H, D, M, C = 2, 2048, 1024, 16, 64, 4, 512
NCORES = 8
R = (B * T) // NCORES  # 512 rows per core
RB = R // 128  # 4 row blocks
FT = F // 128  # 8 feature tiles
MT = (H * D) // 128  # 8 head-pair tiles
MFT = (M * F) // 128  # 32 mlp hidden tiles
KT = T // 128  # 16 key tiles
EPS = 1e-5

_CACHE = {}


def _build_nc():
    import concourse.bass as bass
    import concourse.tile as tile
    from concourse import bacc, mybir
    from concourse.masks import make_identity
    from contextlib import ExitStack

    f32 = mybir.dt.float32
    f16 = mybir.dt.float16
    bf16 = mybir.dt.bfloat16
    AF = mybir.ActivationFunctionType
    OP = mybir.AluOpType

    STOP = int(os.environ.get("STOP_AFTER", "99"))

    nc = bacc.Bacc(
        "TRN2",
        target_bir_lowering=False,
        debug=False,
        enable_asserts=False,
        num_devices=NCORES,
    )

    def dram(name, shape, dt, **kw):
        return nc.dram_tensor(name, shape, dt, **kw).ap()

    x_d = dram("x", [R, F], f32, kind="ExternalInput")
    cond_d = dram("cond", [C], bf16, kind="ExternalInput")
    wmod_d = dram("wmod", [C, 4 * F], bf16, kind="ExternalInput")
    modb_d = dram("modb", [4 * F], bf16, kind="ExternalInput")
    lnv16_d = dram("lnvec16", [6, F], bf16, kind="ExternalInput")
    wq_d = dram("wq", [MT, 128, FT * 128], bf16, kind="ExternalInput")
    qb_d = dram("qbias", [H * D], f32, kind="ExternalInput")
    wkv_d = dram("wkv", [F, 2 * D], bf16, kind="ExternalInput")
    kvb_d = dram("kvbias", [2 * D, 1], f32, kind="ExternalInput")
    # wo pre-paired: [pair, 128(d of even head | d of odd head), F]
    wo_d = dram("wo", [MT, 128, F], bf16, kind="ExternalInput")
    wob_d = dram("wo_bias", [1, F], bf16, kind="ExternalInput")
    w1_d = dram("w1", [MFT, 128, FT * 128], bf16, kind="ExternalInput")
    b1_d = dram("b1", [M * F], f32, kind="ExternalInput")
    w2_d = dram("w2", [M * F, F], bf16, kind="ExternalInput")
    b2_d = dram("b2", [1, F], bf16, kind="ExternalInput")
    y_d = dram("y", [R, F], f32, kind="ExternalOutput")

    groups = [[0, 1, 2, 3], [4, 5, 6, 7]]

    def bcast_row(ap_row):
        # [1, n] DRAM AP -> partition-broadcast [128, n]
        return bass.AP(
            tensor=ap_row.tensor,
            offset=ap_row.offset,
            ap=[[0, 128]] + list(ap_row.ap[-1:]),
        )

    with tile.TileContext(nc) as tc, ExitStack() as ctx:
        # left stack: consts, work, hTp, w1p, hera, aera, wop, [attnp]
        # right stack: wqp, xp, [modtmp], then x1p, g1p
        consts = ctx.enter_context(tc.tile_pool(name="consts", bufs=1))
        work = ctx.enter_context(tc.tile_pool(name="work", bufs=2))
        cm_hTp = tc.tile_pool(name="hTp", bufs=1)
        hTp = cm_hTp.__enter__()
        cm_w1p = tc.tile_pool(name="w1p", bufs=1)
        w1p = cm_w1p.__enter__()
        cm_hera = tc.tile_pool(name="hera", bufs=1)
        hera = cm_hera.__enter__()
        cm_wqp = tc.tile_pool(name="wqp", bufs=1, side="right")
        wqp = cm_wqp.__enter__()
        cm_xp = tc.tile_pool(name="xp", bufs=1, side="right")
        xp = cm_xp.__enter__()
        cm_modtmp = tc.tile_pool(name="modtmp", bufs=1, side="right")
        modtmp = cm_modtmp.__enter__()
        dramp = ctx.enter_context(tc.tile_pool(name="dramp", bufs=1, space="DRAM"))

        # ---------------- constants ----------------
        ident = consts.tile([128, 128], bf16, name="ident")
        make_identity(nc, ident)
        ones16 = consts.tile([1, 128], f16, name="ones16")
        nc.vector.memset(ones16, 1.0)
        onescol = consts.tile([1, 128], bf16, name="onescol")
        nc.vector.memset(onescol, 1.0)
        epst = consts.tile([128, 1], f32, name="epst")
        nc.vector.memset(epst, EPS)
        # ones rows at partitions 64 (even-head denom) and 32 (odd-head denom)
        ones2 = consts.tile([128, 64], f16, name="ones2")
        nc.vector.memset(ones2[64:65, :], 1.0)
        nc.vector.memset(ones2[32:33, :], 1.0)

        # small consts on sync, then wkv (needed by ~35us for the kv proj)
        cond_sb = consts.tile([128, 4], bf16, name="cond_sb")
        nc.sync.dma_start(out=cond_sb, in_=cond_d.rearrange("(a p) -> p a", p=128))
        b1_sb = consts.tile([128, MFT], f32, name="b1_sb")
        nc.sync.dma_start(out=b1_sb, in_=b1_d.rearrange("(mt p) -> p mt", p=128))
        wob_sb = consts.tile([1, F], bf16, name="wob_sb")
        nc.sync.dma_start(out=wob_sb, in_=wob_d)
        b2_sb = consts.tile([1, F], bf16, name="b2_sb")
        nc.sync.dma_start(out=b2_sb, in_=b2_d)
        qb_sb = consts.tile([128, MT], f32, name="qb_sb")
        nc.sync.dma_start(out=qb_sb, in_=qb_d.rearrange("(mt p) -> p mt", p=128))
        kvb_sb = consts.tile([128, 1], f32, name="kvb_sb")
        nc.sync.dma_start(out=kvb_sb, in_=kvb_d)
        lnr = {}
        for r in (0, 1, 4, 5):  # amod_nw/nb, fmod_nw/nb rows at partition 0
            lnr[r] = modtmp.tile([1, F], bf16, name=f"lnr{r}")
            nc.sync.dma_start(out=lnr[r], in_=lnv16_d[r : r + 1, :])
        modb_sb = modtmp.tile([1, 4 * F], bf16, name="modb_sb")
        nc.sync.dma_start(out=modb_sb, in_=modb_d.rearrange("(a f) -> a f", a=1))
        wkv_sb = consts.tile([128, FT, 2 * D], bf16, name="wkv_sb")
        nc.sync.dma_start(
            out=wkv_sb, in_=wkv_d.rearrange("(kt p) n -> p kt n", p=128)
        )

        # wmod chunks: grp0 split across sync (ch0/ch2, interleaved with x)
        # and scalar (ch1/ch3); grp1 on scalar. x row blocks on sync.
        def wm_dma(eng, grp, ch):
            wm = modtmp.tile([128, 2048], bf16, tag="wm", bufs=4, name=f"wm{grp}{ch}")
            eng.dma_start(
                out=wm,
                in_=wmod_d[ch * 128 : (ch + 1) * 128, grp * 2048 : (grp + 1) * 2048],
            )
            return wm

        xs = []

        def x_dma(rb):
            x_rb = xp.tile([128, F], f32, name=f"x{rb}")
            nc.sync.dma_start(out=x_rb, in_=x_d[rb * 128 : (rb + 1) * 128, :])
            xs.append(x_rb)

        wm0 = {}
        wm0[1] = wm_dma(nc.scalar, 0, 1)
        wm0[3] = wm_dma(nc.scalar, 0, 3)
        wm0[0] = wm_dma(nc.sync, 0, 0)
        x_dma(0)
        wm0[2] = wm_dma(nc.sync, 0, 2)
        for rb in (1, 2, 3):
            x_dma(rb)
        wm1 = {ch: wm_dma(nc.scalar, 1, ch) for ch in range(4)}
        # wq split across both queues; lands ~45us, q-proj starts ~50us
        wq_sb = wqp.tile([128, MT, FT * 128], bf16, name="wq_sb")
        for mt in range(4):
            nc.sync.dma_start(out=wq_sb[:, mt, :], in_=wq_d[mt])
        for mt in range(4, MT):
            nc.scalar.dma_start(out=wq_sb[:, mt, :], in_=wq_d[mt])

        # ---------------- phase 0: modulation vectors ----------------
        modv = modtmp.tile([1, 4 * F], f16, name="modv")
        tmpv = modtmp.tile([1, F], f16, name="tmpv")
        modv16 = modtmp.tile([1, 4 * F], f16, name="modv16")
        bc = {}

        def filler(pool, n):
            # dummy matmuls: keep the PE HAM activity window busy
            for _ in range(n):
                wps = pool.tile([128, 512], f32, tag="pmod", bufs=4, name="warm")
                nc.tensor.matmul(wps[:, 0:128], ident, ident, start=True, stop=True)

        def mod_matmuls(ps_pool, grp, wms):
            pms = [
                ps_pool.tile([128, 512], f32, tag="pmod", bufs=4, name=f"pm{j}")
                for j in range(4)
            ]
            for ch in range(4):
                for j in range(4):
                    nc.tensor.matmul(
                        pms[j][0:1, :],
                        cond_sb[:, ch : ch + 1],
                        wms[ch][:, j * 512 : (j + 1) * 512],
                        start=(ch == 0),
                        stop=(ch == 3),
                    )
            with nc.allow_low_precision(reason="f16 modulation vector"):
                for j in range(4):
                    nb = grp * 4 + j
                    nc.vector.tensor_add(
                        out=modv[:, nb * 512 : (nb + 1) * 512],
                        in0=pms[j][0:1, :],
                        in1=modb_sb[:, nb * 512 : (nb + 1) * 512],
                    )

        def finalize_mod(ps_pool, g_off, b_off, nw_row, nb_row, w_name, b_name):
            g_sl = modv[:, g_off : g_off + F]
            b_sl = modv[:, b_off : b_off + F]
            with nc.allow_low_precision(reason="f16 modulation vector"):
                nc.scalar.add(out=g_sl, in_=g_sl, add=1.0)
            with nc.allow_low_precision(reason="f16 staging for PE broadcast"):
                nc.vector.tensor_mul(out=tmpv, in0=g_sl, in1=lnr[nb_row])
                nc.vector.tensor_add(
                    out=modv16[:, b_off : b_off + F], in0=tmpv, in1=b_sl
                )
                nc.vector.tensor_mul(
                    out=modv16[:, g_off : g_off + F], in0=g_sl, in1=lnr[nw_row]
                )
            for off, nm in ((g_off, w_name), (b_off, b_name)):
                bt = consts.tile([128, F], bf16, name=nm)
                for hf in range(2):
                    pb = ps_pool.tile([128, 512], f32, tag="pmod", bufs=4, name="pbc")
                    nc.tensor.matmul(
                        pb,
                        ones16,
                        modv16[:, off + hf * 512 : off + (hf + 1) * 512],
                        start=True,
                        stop=True,
                    )
                    nc.scalar.activation(
                        bt[:, hf * 512 : (hf + 1) * 512], pb, AF.Copy
                    )
                bc[nm] = bt

        cm_ps1 = tc.tile_pool(name="ps1", bufs=1, space="PSUM")
        ps1 = cm_ps1.__enter__()

        filler(ps1, 40)
        mod_matmuls(ps1, 0, wm0)
        finalize_mod(ps1, 0, F, 0, 1, "Wa_bc", "Ba_bc")

        # ---------------- helpers ----------------
        def layer_norm(src, w_bc, b_bc, out_tile, badd_engine):
            """out = LN(src) * w_bc + b_bc ; src [128,F] f32.
            If w_bc is None: out = plain LN(src) (bf16 ok)."""
            stats = work.tile([128, 2, 6], f32, tag="stats", name="stats")
            for sg in range(2):
                nc.vector.bn_stats(
                    out=stats[:, sg, :], in_=src[:, sg * 512 : (sg + 1) * 512]
                )
            mv = work.tile([128, 2], f32, tag="mv", name="mv")
            nc.vector.bn_aggr(out=mv, in_=stats)
            rstd = work.tile([128, 1], f32, tag="rstd", name="rstd")
            nc.scalar.activation(
                out=rstd, in_=mv[:, 1:2], func=AF.Sqrt, bias=epst, scale=1.0
            )
            nc.vector.reciprocal(out=rstd, in_=rstd)
            tgt = out_tile if w_bc is None else work.tile(
                [128, F], f32, tag="xn", bufs=1, name="xn"
            )
            with nc.allow_low_precision(reason="bf16 normalized activations"):
                nc.vector.tensor_scalar(
                    out=tgt,
                    in0=src,
                    scalar1=mv[:, 0:1],
                    scalar2=rstd,
                    op0=OP.subtract,
                    op1=OP.mult,
                )
            if w_bc is None:
                return
            nc.vector.tensor_mul(out=tgt, in0=tgt, in1=w_bc)
            badd_engine.tensor_add(out=out_tile, in0=tgt, in1=b_bc)

        def transpose_to(ps_pool, bufs, hsrc_bf, hT_tiles, rb):
            """hsrc_bf [128,F] bf16 -> hT_tiles[ft][:, rb*128:+128]."""
            for ft in range(FT):
                pt = ps_pool.tile([128, 128], bf16, tag="ptt", bufs=bufs, name="ptt")
                nc.tensor.transpose(
                    pt, hsrc_bf[:, ft * 128 : (ft + 1) * 128], ident
                )
                nc.scalar.activation(
                    out=hT_tiles[ft][:, rb * 128 : (rb + 1) * 128],
                    in_=pt,
                    func=AF.Copy,
                )

        hT = [
            hTp.tile([128, R], bf16, tag=f"hT{ft}", name=f"hT{ft}")
            for ft in range(FT)
        ]

        # ---------------- phase 1: adaLN-1 + attn-LN + transpose ----------------
        h_res = [hera.tile([128, F], f32, name=f"h{rb}") for rb in range(RB)]
        for rb in range(RB):
            # adaLN add on gpsimd: done before the collective reaches the queue
            layer_norm(xs[rb], bc["Wa_bc"], bc["Ba_bc"], h_res[rb], nc.gpsimd)
            hn_bf = work.tile([128, F], bf16, tag="hnbf", bufs=2, name="hn_bf")
            layer_norm(h_res[rb], None, None, hn_bf, None)
            transpose_to(ps1, 2, hn_bf, hT, rb)

        # ---------------- phase 2: kv first (AllGather ASAP) ----------------
        pkv = ps1.tile([128, 512], f32, tag="pkq", bufs=2, name="pkv")
        for kt in range(FT):
            nc.tensor.matmul(
                pkv, wkv_sb[:, kt, :], hT[kt], start=(kt == 0), stop=(kt == FT - 1)
            )
        kvT_sb = work.tile([128, R], bf16, tag="kvT", bufs=1, name="kvT_sb")
        nc.scalar.activation(out=kvT_sb, in_=pkv, func=AF.Identity, bias=kvb_sb)

        kvT_bounce = dramp.tile([2 * D, R], bf16, name="kvT_bounce")
        kvT_all = dramp.tile([4 * 2 * D, R], bf16, name="kvT_all")
        # bounce rides the gpsimd queue: no HWDGE weight traffic in front
        nc.gpsimd.dma_start(out=kvT_bounce, in_=kvT_sb)
        nc.gpsimd.collective_compute(
            "AllGather",
            OP.bypass,
            replica_groups=groups,
            ins=[kvT_bounce[:, :]],
            outs=[kvT_all[:, :]],
        )

        # fmod modulation (wm grp1 has landed by now; tiny PE work) fills
        # a bit of the collective wait, then the q projection.
        mod_matmuls(ps1, 1, wm1)
        finalize_mod(ps1, 2 * F, 3 * F, 4, 5, "Wf_bc", "Bf_bc")
        cm_modtmp.__exit__(None, None, None)
        cm_xp.__exit__(None, None, None)
        cm_aera = tc.tile_pool(name="aera", bufs=1)
        aera = cm_aera.__enter__()
        cm_wop = tc.tile_pool(name="wop", bufs=1)
        wop = cm_wop.__enter__()

        qT = [aera.tile([128, R], bf16, tag=f"qo{mt}", name=f"qT{mt}") for mt in range(MT)]
        for mt in range(MT):
            pq = ps1.tile([128, 512], f32, tag="pkq", bufs=2, name="pq")
            for kt in range(FT):
                nc.tensor.matmul(
                    pq,
                    wq_sb[:, mt, kt * 128 : (kt + 1) * 128],
                    hT[kt],
                    start=(kt == 0),
                    stop=(kt == FT - 1),
                )
            # attention 1/sqrt(D)=0.125 folded into q; attn-LN beta lands
            # via the (pre-scaled) per-partition bias
            nc.scalar.activation(
                out=qT[mt], in_=pq, func=AF.Identity, scale=0.125,
                bias=qb_sb[:, mt : mt + 1],
            )
        cm_wqp.__exit__(None, None, None)

        # weight prefetch dispatched before attention so transfers run
        # under it: wo + w1 first half (scalar q), w1 middle (sync q below)
        wo_sb = wop.tile([128, MT, F], bf16, name="wo_sb")
        for mt in range(MT):
            nc.scalar.dma_start(out=wo_sb[:, mt, :], in_=wo_d[mt])
        W1PRE = 16
        W1MID = 8
        w1_sb = w1p.tile([128, W1PRE, FT * 128], bf16, name="w1_sb")
        for mt in range(W1PRE):
            nc.scalar.dma_start(out=w1_sb[:, mt, :], in_=w1_d[mt])
        w1b_sb = w1p.tile([128, W1MID, FT * 128], bf16, name="w1b_sb")

        # ---------------- phase 3: kT / v_ext assembly ----------------
        kT = aera.tile([128, T], bf16, name="kT")
        for hp in (0, 64):
            for r in range(4):
                nc.sync.dma_start(
                    out=kT[hp : hp + 64, r * R : (r + 1) * R],
                    in_=kvT_all[r * 128 : r * 128 + 64, :],
                )
        v_e = [aera.tile([128, 65], bf16, name=f"ve{kt}") for kt in range(KT)]
        v_o = [aera.tile([128, 128], bf16, name=f"vo{kt}") for kt in range(KT)]
        for kt in range(KT):
            nc.vector.memset(v_e[kt][:, 64:65], 1.0)
            nc.vector.memset(v_o[kt], 0.0)
            nc.vector.memset(v_o[kt][:, 32:33], 1.0)
        filler(ps1, 30)
        for r in range(4):
            vT_sb = work.tile([64, R], bf16, tag="vTs", bufs=2, name="vT_sb")
            nc.sync.dma_start(
                out=vT_sb, in_=kvT_all[r * 128 + 64 : (r + 1) * 128, :]
            )
            for cc in range(4):
                ptv = ps1.tile([128, 128], bf16, tag="ptt", bufs=2, name="ptv")
                nc.tensor.matmul(
                    ptv[:, 0:64],
                    vT_sb[:, cc * 128 : (cc + 1) * 128],
                    ident[0:64, 0:64],
                    is_transpose=True,
                )
                kt = r * 4 + cc
                nc.vector.tensor_copy(out=v_e[kt][:, 0:64], in_=ptv[:, 0:64])
                nc.vector.tensor_copy(out=v_o[kt][:, 64:128], in_=ptv[:, 0:64])

        # w1 middle chunk on the sync queue (idle from here to phase 8):
        # transfers run during attention; the last 8 chunks ring-stream.
        for mt in range(W1MID):
            nc.sync.dma_start(out=w1b_sb[:, mt, :], in_=w1_d[W1PRE + mt])

        cm_ps1.__exit__(None, None, None)

        # ---------------- phase 4: attention ----------------
        # transposed scores [keys, rows]; heads paired (even at PE rows
        # 0-63, odd at rows 64-127) so MM1 row-tiles 2x. exp covers
        # [128,1024] (two kt) per ACT instruction. Software pipeline:
        # MM1 quad k+1 issues before PV quad k; the softmax tail of pair
        # p is emitted inside pair p+1 so the DVE reciprocal and the bcr
        # broadcast matmuls never stall the PE FIFO.
        cm_ps4 = tc.tile_pool(name="ps4", bufs=1, space="PSUM")
        ps4 = cm_ps4.__enter__()
        cm_attnp = tc.tile_pool(name="attnp", bufs=1)
        attnp = cm_attnp.__enter__()

        outT = [
            aera.tile([128, R], bf16, tag=f"qo{mt}", name=f"outT{mt}")
            for mt in range(MT)
        ]

        def tail_a(st):
            # frees po fast: psum reads first, then the slow reciprocal
            mt, po_e, po_o = st
            t_sb = work.tile([128, R], bf16, tag="tsb", bufs=2, name="t_sb")
            nc.vector.tensor_copy(out=t_sb[0:64, :], in_=po_e[0:64, :])
            nc.vector.tensor_copy(out=t_sb[64:128, :], in_=po_o[64:128, :])
            rcpt = work.tile([128, R], f16, tag="rcpt", bufs=2, name="rcpt")
            with nc.allow_low_precision(reason="f16 softmax reciprocal"):
                nc.vector.reciprocal(out=rcpt[64:65, :], in_=po_e[64:65, :])
                nc.vector.reciprocal(out=rcpt[32:33, :], in_=po_o[32:33, :])
            return mt, t_sb, rcpt

        def tail_b(st2):
            mt, t_sb, rcpt = st2
            bcr = ps4.tile([128, 1024], f32, tag="mm1", bufs=2, name="bcr")
            nc.tensor.matmul(
                bcr[0:64, 0:512], ones2[64:65, :], rcpt[64:65, :],
                start=True, stop=True,
            )
            nc.tensor.matmul(
                bcr[64:128, 0:512], ones2[32:33, :], rcpt[32:33, :],
                start=True, stop=True,
            )
            nc.vector.tensor_mul(
                out=outT[mt][0:64, :], in0=t_sb[0:64, :], in1=bcr[0:64, 0:512]
            )
            nc.vector.tensor_mul(
                out=outT[mt][64:128, :], in0=t_sb[64:128, :], in1=bcr[64:128, 0:512]
            )

        pend = None  # completed pair awaiting tail_a
        pend2 = None  # pair awaiting tail_b
        prev_pv = None  # (kt0, pr_e, pr_o, po_e, po_o) awaiting PV

        def emit_pv(st):
            kt0, pr_e, pr_o, po_e, po_o = st
            for i in range(2):
                kt = kt0 + i
                nc.tensor.matmul(
                    po_e[0:65, :],
                    v_e[kt][:, 0:65],
                    pr_e[:, i * 512 : (i + 1) * 512],
                    start=(kt == 0),
                    stop=(kt == KT - 1),
                )
                nc.tensor.matmul(
                    po_o,
                    v_o[kt],
                    pr_o[:, i * 512 : (i + 1) * 512],
                    start=(kt == 0),
                    stop=(kt == KT - 1),
                )

        for mt in range(MT if STOP >= 4 else 0):
            po_e = ps4.tile([128, 512], f32, tag="po", bufs=4, name="po_e")
            po_o = ps4.tile([128, 512], f32, tag="po", bufs=4, name="po_o")
            for ktt in range(8):
                kt0 = 2 * ktt
                ps_e = ps4.tile([128, 1024], f32, tag="mm1", bufs=2, name="ps_e")
                ps_o = ps4.tile([128, 1024], f32, tag="mm1", bufs=2, name="ps_o")
                for i in range(2):
                    ksl = kT[:, (kt0 + i) * 128 : (kt0 + i + 1) * 128]
                    nc.tensor.matmul(
                        ps_e[:, i * 512 : (i + 1) * 512],
                        ksl[0:64, :],
                        qT[mt][0:64, :],
                        start=True,
                        stop=True,
                    )
                    nc.tensor.matmul(
                        ps_o[:, i * 512 : (i + 1) * 512],
                        ksl[64:128, :],
                        qT[mt][64:128, :],
                        start=True,
                        stop=True,
                    )
                pr_e = attnp.tile([128, 1024], bf16, tag="pr", bufs=3, name="pr_e")
                pr_o = attnp.tile([128, 1024], bf16, tag="pr", bufs=3, name="pr_o")
                nc.scalar.activation(out=pr_e, in_=ps_e, func=AF.Exp)
                nc.scalar.activation(out=pr_o, in_=ps_o, func=AF.Exp)
                if prev_pv is not None:
                    emit_pv(prev_pv)
                prev_pv = (kt0, pr_e, pr_o, po_e, po_o)
                if ktt == 1 and pend is not None:
                    pend2 = tail_a(pend)
                    pend = None
                elif ktt == 3 and pend2 is not None:
                    tail_b(pend2)
                    pend2 = None
            emit_pv(prev_pv)
            prev_pv = None
            pend = (mt, po_e, po_o)
        if pend is not None:
            tail_b(tail_a(pend))
            pend = None

        cm_attnp.__exit__(None, None, None)
        cm_ps4.__exit__(None, None, None)

        # ---------------- phase 5+6: out proj -> x1 -> adaLN-2 ----------------
        cm_x1p = tc.tile_pool(name="x1p", bufs=1, side="right")
        x1p = cm_x1p.__enter__()
        cm_ps56 = tc.tile_pool(name="ps56", bufs=1, space="PSUM")
        ps56 = cm_ps56.__enter__()

        x1 = [x1p.tile([128, F], f32, name=f"x1_{rt}") for rt in range(RB)]
        h2T = [
            hTp.tile([128, R], bf16, tag=f"hT{ft}", name=f"h2T{ft}")
            for ft in range(FT)
        ]
        # even and odd heads accumulate into SEPARATE psum tiles (two
        # concurrent PE row-tiles must not write the same psum addresses);
        # the DVE merges them into x1.
        for rt in range(RB if STOP >= 5 else 0):
            px_e = ps56.tile([128, F], f32, tag="pxe", bufs=1, name="px_e")
            px_o = ps56.tile([128, F], f32, tag="pxo", bufs=1, name="px_o")
            rsl = slice(rt * 128, (rt + 1) * 128)
            for mt in range(MT):
                for nh in range(2):
                    fsl = slice(nh * 512, (nh + 1) * 512)
                    nc.tensor.matmul(
                        px_e[:, fsl],
                        outT[mt][0:64, rsl],
                        wo_sb[0:64, mt, fsl],
                        start=(mt == 0),
                        stop=False,
                    )
                    nc.tensor.matmul(
                        px_o[:, fsl],
                        outT[mt][64:128, rsl],
                        wo_sb[64:128, mt, fsl],
                        start=(mt == 0),
                        stop=(mt == MT - 1),
                    )
            # wo bias via ones-row matmul closes the even accumulation
            for nh in range(2):
                fsl = slice(nh * 512, (nh + 1) * 512)
                nc.tensor.matmul(
                    px_e[:, fsl], onescol, wob_sb[:, fsl],
                    start=False, stop=True,
                )
            nc.vector.tensor_add(out=x1[rt], in0=px_e, in1=h_res[rt])
            nc.vector.tensor_add(out=x1[rt], in0=x1[rt], in1=px_o)
            if STOP < 6:
                continue
            h2_bf = work.tile([128, F], bf16, tag="hnbf", bufs=2, name="h2_bf")
            layer_norm(x1[rt], bc["Wf_bc"], bc["Bf_bc"], h2_bf, nc.vector)
            transpose_to(ps56, 2, h2_bf, h2T, rt)

        cm_ps56.__exit__(None, None, None)
        cm_wop.__exit__(None, None, None)
        cm_aera.__exit__(None, None, None)
        cm_hera.__exit__(None, None, None)

        # ---------------- phase 7: mlp1 + gelu ----------------
        cm_ps78 = tc.tile_pool(name="ps78", bufs=1, space="PSUM")
        ps78 = cm_ps78.__enter__()

        w1tail = {}
        for mt in range(W1PRE + W1MID, MFT if STOP >= 7 else 0):
            t = work.tile([128, FT * 128], bf16, tag="w1c", bufs=2, name="w1c")
            nc.sync.dma_start(out=t, in_=w1_d[mt])
            w1tail[mt] = t

        # w2 even chunks stream on the sync queue (no compute there, so
        # ring-slot waits cannot deadlock); odd chunks dispatch on the
        # scalar queue AFTER the gelus (a dispatch before them would wait
        # on phase-8 matmuls that wait on the gelus -> queue deadlock).
        w2c = {}
        for fh in range(2 if STOP >= 8 else 0):
            for kt in range(0, MFT, 2):
                t = work.tile([128, 512], bf16, tag="w2cs", bufs=4, name="w2cs")
                nc.sync.dma_start(
                    out=t,
                    in_=w2_d[kt * 128 : (kt + 1) * 128, fh * 512 : (fh + 1) * 512],
                )
                w2c[(fh, kt)] = t

        cm_g1p = tc.tile_pool(name="g1p", bufs=1, side="right")
        g1p = cm_g1p.__enter__()
        g1T = [g1p.tile([128, R], bf16, name=f"g1T{mt}") for mt in range(MFT)]
        for mt in range(MFT if STOP >= 7 else 0):
            wsrc = (
                w1_sb[:, mt, :] if mt < W1PRE
                else w1b_sb[:, mt - W1PRE, :] if mt < W1PRE + W1MID
                else w1tail[mt]
            )
            pg = ps78.tile([128, 512], f32, tag="pg", bufs=4, name="pg")
            for kt in range(FT):
                nc.tensor.matmul(
                    pg,
                    wsrc[:, kt * 128 : (kt + 1) * 128],
                    h2T[kt],
                    start=(kt == 0),
                    stop=(kt == FT - 1),
                )
            if os.environ.get("SIM_SAFE"):
                nc.scalar.activation(out=g1T[mt], in_=pg, func=AF.Exp)
            else:
                nc.scalar.activation(
                    out=g1T[mt], in_=pg, func=AF.Gelu,
                    bias=b1_sb[:, mt : mt + 1], scale=1.0,
                )

        cm_w1p.__exit__(None, None, None)
        cm_hTp.__exit__(None, None, None)

        # ---------------- phase 8: mlp2 + residual -> y ----------------
        for fh in range(2 if STOP >= 8 else 0):
            for kt in range(1, MFT, 2):
                t = work.tile([128, 512], bf16, tag="w2ca", bufs=4, name="w2ca")
                nc.scalar.dma_start(
                    out=t,
                    in_=w2_d[kt * 128 : (kt + 1) * 128, fh * 512 : (fh + 1) * 512],
                )
                w2c[(fh, kt)] = t

        if STOP < 8:
            for rt in range(RB):
                yh = work.tile([128, F], f32, tag="ydummy", bufs=2, name="ydummy")
                nc.vector.memset(yh, 0.0)
                nc.sync.dma_start(out=y_d[rt * 128 : (rt + 1) * 128, :], in_=yh)
        for fh in range(2 if STOP >= 8 else 0):
            pf = {}
            for rt in range(RB):
                pf[rt] = ps78.tile([128, 512], f32, tag="pg", bufs=4, name=f"pf{rt}")
            for kt in range(MFT):
                for rt in range(RB):
                    nc.tensor.matmul(
                        pf[rt],
                        g1T[kt][:, rt * 128 : (rt + 1) * 128],
                        w2c[(fh, kt)],
                        start=(kt == 0),
                        stop=False,
                    )
            fsl = slice(fh * 512, (fh + 1) * 512)
            for rt in range(RB):
                nc.tensor.matmul(
                    pf[rt], onescol, b2_sb[:, fsl], start=False, stop=True
                )
            for rt in range(RB):
                yh = work.tile([128, 512], f32, tag="yh", bufs=2, name="yh")
                nc.vector.tensor_add(out=yh, in0=pf[rt], in1=x1[rt][:, fsl])
                nc.sync.dma_start(out=y_d[rt * 128 : (rt + 1) * 128, fsl], in_=yh)

        cm_g1p.__exit__(None, None, None)
        cm_x1p.__exit__(None, None, None)
        cm_ps78.__exit__(None, None, None)

    nc.compile()
    return nc


def _prep_in_maps(inputs):
    f32 = np.float32
    wmod = np.concatenate(
        [inputs["amod_gw"], inputs["amod_bw"], inputs["fmod_gw"], inputs["fmod_bw"]],
        axis=1,
    ).astype(BF16)
    modb = np.concatenate(
        [inputs["amod_gb"], inputs["amod_bb"], inputs["fmod_gb"], inputs["fmod_bb"]]
    ).astype(BF16)
    lnvec = np.stack(
        [
            inputs["amod_nw"],
            inputs["amod_nb"],
            inputs["attn_nw"],
            inputs["attn_nb"],
            inputs["fmod_nw"],
            inputs["fmod_nb"],
        ]
    ).astype(f32)
    # fold the attention-internal LN gamma/beta into wq/wkv
    anw = np.asarray(inputs["attn_nw"]).astype(f32)
    anb = np.asarray(inputs["attn_nb"]).astype(f32)
    wq_f = np.asarray(inputs["wq"]).astype(f32)
    wkv_f = np.asarray(inputs["wkv"]).astype(f32)
    wq_eff = (wq_f * anw[:, None]).astype(BF16)
    wkv_eff = (wkv_f * anw[:, None]).astype(BF16)
    qbias = (anb @ wq_f).astype(f32) * 0.125  # qT copy applies scale=0.125
    kvbias = (anb @ wkv_f).astype(f32).reshape(2 * D, 1)
    wq_t = np.ascontiguousarray(
        wq_eff.reshape(FT, 128, MT, 128)
        .transpose(2, 1, 0, 3).reshape(MT, 128, FT * 128)
    )
    w1_t = np.ascontiguousarray(
        np.asarray(inputs["w1"]).astype(BF16).reshape(FT, 128, MFT, 128)
        .transpose(2, 1, 0, 3).reshape(MFT, 128, FT * 128)
    )
    # wo [H*D, F] -> pair layout [MT, 128, F]: partitions 0-63 = head 2i,
    # 64-127 = head 2i+1.
    wo = np.asarray(inputs["wo"]).astype(BF16).reshape(H, D, F)
    wo_t = np.ascontiguousarray(
        np.stack([np.concatenate([wo[2 * i], wo[2 * i + 1]], 0) for i in range(MT)])
    )
    shared = dict(
        wmod=wmod,
        modb=modb,
        lnvec16=lnvec.astype(BF16),
        wq=wq_t,
        qbias=qbias,
        wkv=wkv_eff,
        kvbias=kvbias,
        wo=wo_t,
        wo_bias=np.asarray(inputs["wo_b"]).astype(BF16).reshape(1, F),
        w1=w1_t,
        b1=np.asarray(inputs["b1"]).astype(f32),
        w2=np.asarray(inputs["w2"]).astype(BF16),
        b2=np.asarray(inputs["b2"]).astype(BF16).reshape(1, F),
    )
    x = np.asarray(inputs["x"]).astype(f32)
    cond = np.asarray(inputs["cond"]).astype(BF16)
    in_maps = []
    for c in range(NCORES):
        b, r0 = c // 4, (c % 4) * R
        m = dict(shared)
        m["x"] = np.ascontiguousarray(x[b, r0 : r0 + R, :])
        m["cond"] = np.ascontiguousarray(cond[b])
        in_maps.append(m)
    return in_maps


def _run(inputs, trace=False):
    from concourse.bass_utils import run_bass_kernel_spmd

    if "nc" not in _CACHE:
        _CACHE["nc"] = _build_nc()
    nc = _CACHE["nc"]
    in_maps = _prep_in_maps(inputs)
    res = run_bass_kernel_spmd(
        nc, in_maps, core_ids=list(range(NCORES)), trace=trace
    )
    y = np.empty((B, T, F), np.float32)
    for c in range(NCORES):
        b, r0 = c // 4, (c % 4) * R
        y[b, r0 : r0 + R, :] = res.results[c]["y"]
    return y, res


def kernel(**inputs) -> np.ndarray:
    y, _ = _run(inputs, trace=False)
    return y


if __name__ == "__main__":
    _build_nc()
    print("build OK")
